# revision 43
# baseline (speedup 1.0000x reference)
"""MultiHeadAttention Trainium2 kernel (8 NeuronCores).

Reference computation (B=4, T=2048, D=512, H=8, head_dim=64):
    q = split_heads(queries @ Wq + bq); k, v likewise
    wei = softmax(q k^T / sqrt(512) + (-1e9) * mask)   # mask: causal
    out = merge_heads(wei @ v) @ Wo + bo

Sharding: core = 2*b + g  (b in 0..3 batches, g in 0..1 groups of 4 heads).
Each core computes attention for its batch with its 4 heads plus a partial
output projection through its half of Wo's rows. Host sums the two partials
per batch and adds bo + bv @ Wo (the value bias is separable: softmax rows
sum to 1, so V += bv shifts each head's output by exactly bv).

Device-side layout: everything transposed (feature dim on partitions):
    Q^T, K^T : [64, T] per head, heads of a pair at partition offsets 0/64
    S^T blocks [128 k, q] via lhsT=K^T_j, rhs=Q^T; the pair's two K=64
        matmuls land on PE row groups 0/64 and run concurrently
    P^T = exp(scale * S^T) on ScalarE (both heads in one ACTIVATE),
        causal handled by skipping dead ranges + one 0/1 lower-triangular
        multiply on the diagonal 128x128 windows
    O^T[65, q] accumulates lhsT=[V | ones]; row 64 = softmax denominators,
        reciprocal'd on VectorE straight out of PSUM, then broadcast across
        64 partitions via a DRAM-bounce DMA (gpsimd queue)
    Y[t, 512] partial = stacked normalized O^T pairs against Wo_g rows

Schedule: ScalarE exp is the bottleneck engine (~92us of ACTIVATE), so the
emission order interleaves everything else into the attention j-loops:
pair-1 Q/K projections and V projections ride inside pair-0's attention,
the output projection rides inside pair-1's attention. All projection
PSUM->SBUF copies run on VectorE (bias added against a pre-materialized
per-partition bias tile) so ScalarE does exp only. Input DMAs are issued
column-group-first so the first attention chunk's Q/K/V are computable
~6.5us in, with dummy warm-up matmuls opening the PE HAM clock gate.
"""

import sys

if "/opt/trn_rl_repo" not in sys.path:
    sys.path.insert(0, "/opt/trn_rl_repo")

import numpy as np
import ml_dtypes

B, T, D, H = 4, 2048, 512, 8
HPG = 4                 # heads per group (per core)
HD = 64                 # head dim
DG = HPG * HD           # 256 feature dims per group
N_CORES = 8
SM_SCALE = float(D) ** -0.5   # module scales by full d_k = 512

_BF16 = ml_dtypes.bfloat16
_F8 = ml_dtypes.float8_e4m3
# q/k/v weights are pre-scaled by WS on the host so their ~N(0, 0.02^2)
# entries land in fp8e4's normal range; the projection drains divide it
# back out (activation scale= for q/k, Wo/WS host-side for v)
WS = 128.0

# Schraudolph exp2 in bf16-bit space: int16(SCH_A * s + SCH_B) bitcast to
# bf16 approximates exp(SM_SCALE * s) within +-3% (shift 0.043 balances the
# piecewise-linear mantissa error; verified on-device)
SCH_A = float(128.0 * SM_SCALE * np.log2(np.e))
SCH_B = float(127 * 128 - 0.043 * 128)

_compiled = None


def _build():
    import concourse.bass as bass
    import concourse.bacc as bacc
    import concourse.tile as tile
    import concourse.mybir as mybir

    f32 = mybir.dt.float32
    bf16 = mybir.dt.bfloat16
    f8 = mybir.dt.float8e4
    DR = mybir.MatmulPerfMode.DoubleRow
    Exp = mybir.ActivationFunctionType.Exp
    Ident = mybir.ActivationFunctionType.Identity
    mult = mybir.AluOpType.mult
    add = mybir.AluOpType.add

    nc = bacc.Bacc("TRN2", target_bir_lowering=False, debug=False,
                   num_devices=N_CORES)

    # x tensors are host-pre-swizzled to [128, cg, dc, 512] so each
    # column-group load is one dma with 4KB-contiguous per-partition runs
    # (source AND dest), and each cg lands in its OWN tile so the first
    # projection only waits on its own cg's DMA (tile deps are whole-tile)
    xq = nc.dram_tensor("xq_t", [128, 4, 4, 512], f8,
                        kind="ExternalInput").ap()
    xk = nc.dram_tensor("xk_t", [128, 4, 4, 512], f8,
                        kind="ExternalInput").ap()
    # v path stays bf16: near-uniform softmax rows average V down ~30x, so
    # per-entry V quantization noise (~3.6% in fp8) passes straight through
    # to the output; q/k fp8 only costs ~0.4% after the exp
    xv = nc.dram_tensor("xv_t", [128, 4, 4, 512], bf16,
                        kind="ExternalInput").ap()
    wq = nc.dram_tensor("wq", [128, 4 * DG], f8, kind="ExternalInput").ap()
    wk = nc.dram_tensor("wk", [128, 4 * DG], f8, kind="ExternalInput").ap()
    wv = nc.dram_tensor("wv", [128, 4 * DG], bf16,
                        kind="ExternalInput").ap()
    wo = nc.dram_tensor("wo", [128, 2 * D], bf16, kind="ExternalInput").ap()
    # q and k biases in one tensor: [128, (q|k) x pc] — a single tiny DMA
    bqk = nc.dram_tensor("bqk", [128, 4], f32, kind="ExternalInput").ap()
    y = nc.dram_tensor("y", [T, D], bf16, kind="ExternalOutput").ap()

    NT = T // 512        # 4 t/q chunks of 512
    NB = T // 128        # 16 t/k blocks of 128
    ND = D // 128        # 4 contraction chunks over D

    with tile.TileContext(nc) as tc:
        with (
            tc.tile_pool(name="const", bufs=1) as const,
            tc.tile_pool(name="pt", bufs=10) as ppool,
            tc.tile_pool(name="rc", bufs=4) as rcpool,
            tc.tile_pool(name="ysb", bufs=4) as ypool,
            tc.tile_pool(name="psA", bufs=2, space="PSUM") as psA,
            tc.tile_pool(name="psO", bufs=2, space="PSUM") as psO,
        ):
            # ---- input DMAs: per-cg tiles, critical loads first ----------
            def load_w(dram, name, eng, dt):
                t = const.tile([128, ND, DG], dt, tag=name)
                eng.dma_start(
                    out=t[:], in_=dram.rearrange("p (c m) -> p c m", c=ND))
                return t

            def load_x_cg(dram, name, cg, eng, dt):
                # one tile per (tensor, cg): the dependent projections wait
                # only on this one dma (contiguous per-partition runs)
                t = const.tile([128, ND, 512], dt, tag=f"{name}{cg}")
                eng.dma_start(out=t[:], in_=dram[:, cg])
                return t

            def load_x_cg23(dram, name, eng, dt):
                # trailing two column groups in one dma / one tile
                t = const.tile([128, 2, ND, 512], dt, tag=f"{name}23")
                eng.dma_start(out=t[:], in_=dram[:, 2:4])
                return t[:, 0], t[:, 1]

            # the dma issue itself costs the issuing engine ~0.65us, so the
            # startup-critical loads are spread across otherwise-idle
            # engine queues (tensor issues xq0 before its warm-ups, scalar
            # issues wq before its first activation)
            xq_sb = [None] * NT
            xk_sb = [None] * NT
            xv_sb = [None] * NT
            xq_sb[0] = load_x_cg(xq, "xq", 0, nc.scalar, f8)
            xk_sb[0] = load_x_cg(xk, "xk", 0, nc.gpsimd, f8)
            wq_sb = load_w(wq, "wq", nc.scalar, f8)
            wk_sb = load_w(wk, "wk", nc.gpsimd, f8)
            bqk_sb = const.tile([128, 4], f32, tag="bqk")
            nc.sync.dma_start(out=bqk_sb[:], in_=bqk)
            bq_sb = bqk_sb[:, 0:2]
            bk_sb = bqk_sb[:, 2:4]
            wv_sb = load_w(wv, "wv", nc.gpsimd, bf16)
            xv_sb[0] = load_x_cg(xv, "xv", 0, nc.sync, bf16)
            xv_sb[1] = load_x_cg(xv, "xv", 1, nc.sync, bf16)
            xq_sb[1] = load_x_cg(xq, "xq", 1, nc.sync, f8)
            xk_sb[1] = load_x_cg(xk, "xk", 1, nc.gpsimd, f8)
            xq_sb[2], xq_sb[3] = load_x_cg23(xq, "xq", nc.sync, f8)
            xk_sb[2], xk_sb[3] = load_x_cg23(xk, "xk", nc.gpsimd, f8)
            xv_sb[2], xv_sb[3] = load_x_cg23(xv, "xv", nc.sync, bf16)
            wo_sb = const.tile([128, 2, D], bf16, tag="wo")
            nc.sync.dma_start(out=wo_sb[:],
                              in_=wo.rearrange("p (c n) -> p c n", c=2))

            # warm-up during the input-DMA prologue (HAM clock gate)
            warm = const.tile([128, 512], bf16, tag="warm")
            nc.vector.memset(warm[:], 0.0)
            wps = psA.tile([128, 1024], f32, tag="ps", name="wps")
            for _ in range(12):
                nc.tensor.matmul(wps[:, :512], lhsT=warm[:, :128],
                                 rhs=warm[:], start=True, stop=True)
            wexp = const.tile([1, 2], bf16, tag="wexp")
            nc.scalar.activation(wexp[:], warm[0:1, 0:2], Exp)

            # 0/1 lower-triangular for both heads: tri2[k, hh, q] = (q >= k)
            tri2 = const.tile([128, 2, 128], bf16, tag="tri2")
            nc.gpsimd.memset(tri2[:], 1.0)
            for hh in range(2):
                nc.gpsimd.affine_select(
                    out=tri2[:, hh, :], in_=tri2[:, hh, :],
                    compare_op=mybir.AluOpType.is_ge, fill=0.0,
                    base=0, pattern=[[1, 128]], channel_multiplier=-1)

            qT = const.tile([128, 2, T], bf16, tag="qT")
            kT = const.tile([128, 2, T], bf16, tag="kT")
            # PV lhsT = [V_head | 64 ones-cols]: the PE replicates the
            # softmax denominator across PSUM partitions 64..127, so the
            # normalize needs no cross-partition broadcast at all
            vA = const.tile([128, NB, HPG, 2 * HD], bf16, tag="vA")
            nc.vector.memset(vA[:, :, :, HD:], 1.0)
            oTn = const.tile([128, 2, T], bf16, tag="oTn")

            # ---- side-work groups (emitted inside attention j-loops) ----
            def qk2(pc, c):
                # Q^T and K^T chunk c for partition-chunk pc (one 2-bank slot)
                ps = psA.tile([128, 2, 512], f32, tag="ps", name="ps")
                items = ((qT, wq_sb, xq_sb[c], bq_sb),
                         (kT, wk_sb, xk_sb[c], bk_sb))
                for i, (dst, w_sb, x_sb, bsb) in enumerate(items):
                    # fp8 DoubleRow: two 128-row D-chunks contract per matmul
                    for dp in range(ND // 2):
                        nc.tensor.matmul(
                            ps[:, i, :],
                            lhsT=w_sb[:, 2 * dp:2 * dp + 2,
                                      128 * pc:128 * (pc + 1)],
                            rhs=x_sb[:, 2 * dp:2 * dp + 2, :],
                            perf_mode=DR,
                            start=(dp == 0), stop=(dp == ND // 2 - 1),
                            skip_group_check=True)
                # drain on VectorE: one fused (x * 1/WS) + bias per half —
                # keeps the ScalarE queue pure exp
                for i, (dst, w_sb, x_sb, bsb) in enumerate(items):
                    nc.vector.tensor_scalar(
                        out=dst[:, pc, 512 * c:512 * (c + 1)],
                        in0=ps[:, i, :], scalar1=1.0 / WS,
                        scalar2=bsb[:, pc:pc + 1], op0=mult, op1=add)

            def v_quad(tb0):
                # V rows for 4 t-blocks (one 2-bank slot)
                ps = psA.tile([128, 1024], f32, tag="ps", name="ps")
                x_sb = xv_sb[tb0 // 4]
                for i in range(4):
                    for dc in range(ND):
                        nc.tensor.matmul(
                            ps[:, 256 * i:256 * (i + 1)],
                            lhsT=x_sb[:, dc, 128 * i:128 * (i + 1)],
                            rhs=wv_sb[:, dc, :],
                            start=(dc == 0), stop=(dc == ND - 1),
                            skip_group_check=True)
                nc.vector.tensor_copy(
                    vA[:, tb0:tb0 + 4, :, 0:HD],
                    ps.rearrange("p (t h d) -> p t h d", t=4, h=HPG))

            def o_double(tb0, tail=False):
                # output projection for 2 t-blocks (one 2-bank slot)
                yp = psA.tile([128, 1024], f32, tag="ps", name="ps")
                for i in range(2):
                    for pair in range(2):
                        nc.tensor.matmul(
                            yp[:, 512 * i:512 * (i + 1)],
                            lhsT=oTn[:, pair,
                                     128 * (tb0 + i):128 * (tb0 + i + 1)],
                            rhs=wo_sb[:, pair, :],
                            start=(pair == 0), stop=(pair == 1),
                            skip_group_check=True)
                ysb = ypool.tile([128, 1024], bf16, tag="ysb")
                # drain on ScalarE (partition-aligned PSUM read): VectorE
                # carries the rest of the elementwise load
                nc.scalar.activation(ysb[:], yp[:],
                                     mybir.ActivationFunctionType.Copy)
                for i in range(2):
                    nc.sync.dma_start(
                        out=y[128 * (tb0 + i):128 * (tb0 + i + 1), :],
                        in_=ysb[:, 512 * i:512 * (i + 1)])

            # PV runs one exp-step behind, across chunk/pair boundaries:
            # while ScalarE computes exp(j), the PE has S(j+1) AND PV(j-1)
            # queued — the exp stream never waits on a PV drain
            pend_pv = []

            def emit_pv():
                while pend_pv:
                    oT_, a_, j_, pair_, rhs_, first, last = pend_pv.pop(0)
                    for hh in range(2):
                        nc.tensor.matmul(
                            oT_[:, hh, a_:],
                            lhsT=vA[:, j_, 2 * pair_ + hh, :],
                            rhs=rhs_[hh],
                            start=first, stop=last,
                            skip_group_check=True)
                        # partitions 0..63: O^T rows; 64..127: the softmax
                        # denominator replicated 64x (ones-cols of vA)

            # ---- attention ------------------------------------------------
            def attention_qc(pair, qc, side):
                """Emit one q-chunk of attention. Returns deferred normalize
                closures (one per head) — emitted later, inside the NEXT
                chunk's j-loop, so their wait on the rc broadcast DMA never
                head-of-line-blocks the Vector queue (which would stall psA
                slot releases and starve ScalarE)."""
                q0 = 512 * qc
                n_j = 4 * qc + 4
                oT = psO.tile([128, 2, 512], f32, tag="oT", name="oT")
                # diagonal windows first: their exp->mask->PV chain crosses
                # an extra engine, so keep it off the chunk tail where it
                # would delay the next chunk's S matmuls in the PE FIFO
                order = list(range(4 * qc, n_j)) + list(range(4 * qc))

                for idx, j in enumerate(order):
                    a = max(0, 128 * j - q0)
                    sT = psA.tile([128, 2, 512], f32, tag="ps", name="sT")
                    for hh in range(2):
                        o = 64 * hh
                        nc.tensor.matmul(
                            sT[:, hh, a:],
                            lhsT=kT[o:o + 64, pair, 128 * j:128 * (j + 1)],
                            rhs=qT[o:o + 64, pair, q0 + a:q0 + 512],
                            start=True, stop=True, skip_group_check=True)
                    sch = 128 * j < q0 and j % 3 == 1
                    if sch:
                        # Schraudolph exp2: bf16 bits of 2^t are linear in t
                        # (+-3% after the balancing shift) — one VectorE op
                        # offloads this block's exp from busy ScalarE
                        pTi = ppool.tile([128, 2, 512], mybir.dt.int16,
                                         tag="pt")
                        nc.vector.tensor_scalar(
                            out=pTi[:], in0=sT[:], scalar1=SCH_A,
                            scalar2=SCH_B, op0=mult, op1=add)
                        pv_rhs = [pTi[:, hh, :].bitcast(bf16)
                                  for hh in range(2)]
                    else:
                        pT = ppool.tile([128, 2, 512], bf16, tag="pt")
                        nc.scalar.activation(pT[:, :, a:], sT[:, :, a:], Exp,
                                             scale=SM_SCALE)
                        if 128 * j >= q0:  # diagonal window, both heads
                            nc.gpsimd.tensor_tensor(
                                pT[:, :, a:a + 128], pT[:, :, a:a + 128],
                                tri2[:], mult)
                        pv_rhs = [pT[:, hh, a:] for hh in range(2)]
                    emit_pv()
                    pend_pv.append((oT, a, j, pair, pv_rhs,
                                    idx == 0, idx == n_j - 1))
                    if idx < len(side) and side[idx] is not None:
                        side[idx]()
                emit_pv()  # drain this chunk's last PV before its boundary
                # softmax denominator work is ALL deferred into the next
                # chunk's j-loop: the reciprocal waits on this chunk's last
                # PV matmul — emitted at a boundary it would head-of-line-
                # block the Vector queue, stalling psA slot releases and
                # starving ScalarE.
                rc = rcpool.tile([64, 2, 512], f32, tag="rc")

                def fin0():
                    # denominators sit pre-replicated on PSUM partitions
                    # 64..127 (ones-block of vA). Partition-shifted copy on
                    # ScalarE (keeps the Vector queue free at the chunk
                    # boundary), then reciprocal on VectorE (custom-DVE ops
                    # and mismatched in0/in1 offsets don't shift on hw).
                    nc.scalar.activation(rc[:], oT[HD:, :, :],
                                         mybir.ActivationFunctionType.Copy)
                    nc.vector.reciprocal_approx_fast(rc[:], rc[:])

                def norm(hh):
                    nc.vector.tensor_tensor(
                        oTn[64 * hh:64 * hh + 64, pair, q0:q0 + 512],
                        oT[0:HD, hh, :], rc[:, hh, :], mult)
                return [fin0, lambda: norm(0), lambda: norm(1)]

            def place(n_j, assign):
                side = [None] * n_j
                for slot, fn in assign:
                    assert side[slot] is None and slot < n_j
                    side[slot] = fn
                return side

            pend = []  # deferred boundary closures from the previous qc

            def run_qc(pair, qc, extras):
                # prev chunk's srow/recip/DMA at slot 0, normalizes at 4,6
                # (2,3 when the chunk is short); extras carry explicit slots
                nonlocal pend
                n_j = 4 * qc + 4
                assign = []
                if pend:
                    f0, n0, n1 = pend
                    if n_j >= 8:
                        assign += [(0, f0), (4, n0), (6, n1)]
                    else:
                        assign += [(0, f0), (2, n0), (3, n1)]
                assign += extras
                pend = attention_qc(pair, qc, place(n_j, assign))

            qk2(0, 0)
            v_quad(0)
            run_qc(0, 0, [(0, lambda: v_quad(4)), (1, lambda: qk2(0, 1))])
            run_qc(0, 1, [(1, lambda: qk2(0, 2)), (3, lambda: v_quad(8)),
                          (5, lambda: qk2(1, 0))])
            run_qc(0, 2, [(1, lambda: qk2(0, 3)), (3, lambda: v_quad(12)),
                          (5, lambda: qk2(1, 1)), (7, lambda: qk2(1, 2))])
            run_qc(0, 3, [(1, lambda: qk2(1, 3))])
            run_qc(1, 0, [])
            run_qc(1, 1, [])
            run_qc(1, 2, [(1, lambda: o_double(0)), (3, lambda: o_double(2)),
                          (8, lambda: o_double(4)),
                          (10, lambda: o_double(6))])
            run_qc(1, 3, [(8, lambda: o_double(8)),
                          (10, lambda: o_double(10))])
            emit_pv()
            for fn in pend:
                fn()
            o_double(12, tail=True)
            o_double(14)

    nc.compile()
    return nc


def _get_compiled():
    global _compiled
    if _compiled is None:
        _compiled = _build()
    return _compiled


def _reference_fallback(queries, keys, values, mask, Wq, bq, Wk, bk, Wv, bv,
                        Wo, bo):
    def split_heads(x):
        b, t, c = x.shape
        return x.reshape(b, t, H, c // H).transpose(0, 2, 1, 3)

    q = split_heads(queries @ Wq + bq)
    k = split_heads(keys @ Wk + bk)
    v = split_heads(values @ Wv + bv)
    wei = np.einsum("bhqd,bhkd->bhqk", q, k) * SM_SCALE
    wei = wei + (-1e9) * mask
    wei = wei - wei.max(axis=-1, keepdims=True)
    wei = np.exp(wei)
    wei = wei / wei.sum(axis=-1, keepdims=True)
    out = np.einsum("bhqk,bhkd->bhqd", wei, v)
    out = out.transpose(0, 2, 1, 3).reshape(queries.shape[0],
                                            queries.shape[1], D)
    return (out @ Wo + bo).astype(np.float32)


def _xprep(x, dtype):
    """[T, D] -> transposed [128, cg, dc, 512]: per (partition, cg) the
    4 dc-chunks are contiguous DMA runs."""
    xt = np.asarray(x).T  # [D, T]
    return np.ascontiguousarray(
        xt.reshape(4, 128, 4, 512).transpose(1, 2, 0, 3)
    ).astype(dtype)


def _wprep(w, scale, dtype):
    """[D, DG] -> partition-major [128, ND*DG] in `dtype`, pre-scaled."""
    w = np.asarray(w) * scale
    n = w.shape[1]
    return np.ascontiguousarray(
        w.reshape(-1, 128, n).swapaxes(0, 1).reshape(128, -1)).astype(dtype)


def make_in_maps(queries, keys, values, Wq, bq, Wk, bk, Wv, Wo):
    in_maps = []
    for core in range(N_CORES):
        b, g = core // 2, core % 2
        sl = slice(g * DG, (g + 1) * DG)
        in_maps.append({
            "xq_t": _xprep(queries[b], _F8),
            "xk_t": _xprep(keys[b], _F8),
            "xv_t": _xprep(values[b], _BF16),
            "wq": _wprep(Wq[:, sl], WS, _F8),
            "wk": _wprep(Wk[:, sl], WS, _F8),
            "wv": _wprep(Wv[:, sl], 1.0, _BF16),
            "wo": _wprep(Wo[sl, :], 1.0, _BF16),
            "bqk": np.ascontiguousarray(np.concatenate([
                np.asarray(bq, np.float32)[sl].reshape(2, 128).T,
                np.asarray(bk, np.float32)[sl].reshape(2, 128).T,
            ], axis=1)),
        })
    return in_maps


def kernel(queries, keys, values, mask, Wq, bq, Wk, bk, Wv, bv, Wo, bo):
    queries = np.asarray(queries, np.float32)
    keys = np.asarray(keys, np.float32)
    values = np.asarray(values, np.float32)
    Wq, Wk, Wv, Wo = (np.asarray(w, np.float32) for w in (Wq, Wk, Wv, Wo))
    bq, bk, bv, bo = (np.asarray(v_, np.float32) for v_ in (bq, bk, bv, bo))
    mask2d = np.asarray(mask, np.float32).reshape(T, T)
    causal = np.triu(np.ones((T, T), np.float32), k=1)
    if not np.array_equal(mask2d, causal):
        return _reference_fallback(queries, keys, values,
                                   np.asarray(mask, np.float32),
                                   Wq, bq, Wk, bk, Wv, bv, Wo, bo)

    from concourse.bass_utils import run_bass_kernel_spmd

    nc = _get_compiled()
    in_maps = make_in_maps(queries, keys, values, Wq, bq, Wk, bk, Wv, Wo)
    res = run_bass_kernel_spmd(nc, in_maps, list(range(N_CORES)))
    out = np.zeros((B, T, D), np.float32)
    for core in range(N_CORES):
        out[core // 2] += res.results[core]["y"].astype(np.float32)
    out += bo + bv @ Wo   # value bias is separable (softmax rows sum to 1)
    return out



# revision 45
# speedup vs baseline: 1.0471x; 1.0471x over previous
"""MultiHeadAttention Trainium2 kernel (8 NeuronCores).

Reference computation (B=4, T=2048, D=512, H=8, head_dim=64):
    q = split_heads(queries @ Wq + bq); k, v likewise
    wei = softmax(q k^T / sqrt(512) + (-1e9) * mask)   # mask: causal
    out = merge_heads(wei @ v) @ Wo + bo

Sharding: core = 2*b + g  (b in 0..3 batches, g in 0..1 groups of 4 heads).
Each core computes attention for its batch with its 4 heads plus a partial
output projection through its half of Wo's rows. Host sums the two partials
per batch and adds bo + bv @ Wo (the value bias is separable: softmax rows
sum to 1, so V += bv shifts each head's output by exactly bv).

Device-side layout: everything transposed (feature dim on partitions):
    Q^T, K^T : [64, T] per head, heads of a pair at partition offsets 0/64;
        projected with fp8e4 DoubleRow matmuls (weights host-prescaled by WS
        into fp8's normal range; the drain multiplies 1/WS back).  V stays
        bf16: near-uniform softmax rows average V ~30x down, so fp8's ~3.6%
        per-entry noise would pass straight through to the output, while
        q/k fp8 costs only ~0.4% after the exp.
    S^T blocks [128 k, q] via lhsT=K^T_j, rhs=Q^T; the pair's two K=64
        matmuls land on PE row groups 0/64 and run concurrently
    P^T = exp(scale * S^T) on ScalarE (both heads in one ACTIVATE),
        causal handled by skipping dead ranges + one 0/1 lower-triangular
        multiply on the diagonal 128x128 windows; every third
        below-diagonal block moves to VectorE via a Schraudolph exp2 in
        bf16-bit space (int16 mult-add, then bitcast)
    O^T[128, q] accumulates lhsT=[V | 64 ones-cols]: partitions 64..127
        receive the softmax denominator pre-replicated 64x by the PE
        itself, so normalization is one partition-shifted VectorE copy +
        reciprocal + multiply — no cross-partition broadcast DMA
    Y[t, 512] partial = stacked normalized O^T pairs against Wo_g rows,
        drained PSUM->SBUF on ScalarE (activation Copy)

Schedule: the emission order interleaves all projections into the
attention j-loops: pair-1 Q/K projections and V projections ride inside
pair-0's attention, the output projection rides inside pair-1's attention.
Inputs are host-swizzled to [128, cg, dc, 512] with one SBUF tile per
column group, so each projection waits only on its own cg's DMA
(tile deps are whole-tile); dma issues (~0.65us of issuing-engine time
each) are spread over the sync, gpsimd AND scalar queues; dummy warm-up
matmuls open the PE HAM clock gate while inputs stream in.
"""

import sys

if "/opt/trn_rl_repo" not in sys.path:
    sys.path.insert(0, "/opt/trn_rl_repo")

import numpy as np
import ml_dtypes

B, T, D, H = 4, 2048, 512, 8
HPG = 4                 # heads per group (per core)
HD = 64                 # head dim
DG = HPG * HD           # 256 feature dims per group
N_CORES = 8
SM_SCALE = float(D) ** -0.5   # module scales by full d_k = 512

_BF16 = ml_dtypes.bfloat16
_F8 = ml_dtypes.float8_e4m3
# q/k/v weights are pre-scaled by WS on the host so their ~N(0, 0.02^2)
# entries land in fp8e4's normal range; the projection drains divide it
# back out (activation scale= for q/k, Wo/WS host-side for v)
WS = 128.0

# Schraudolph exp2 in bf16-bit space: int16(SCH_A * s + SCH_B) bitcast to
# bf16 approximates exp(SM_SCALE * s) within +-3% (shift 0.043 balances the
# piecewise-linear mantissa error; verified on-device)
SCH_A = float(128.0 * SM_SCALE * np.log2(np.e))
SCH_B = float(127 * 128 - 0.043 * 128)

_compiled = None


def _build():
    import concourse.bass as bass
    import concourse.bacc as bacc
    import concourse.tile as tile
    import concourse.mybir as mybir

    f32 = mybir.dt.float32
    bf16 = mybir.dt.bfloat16
    f8 = mybir.dt.float8e4
    DR = mybir.MatmulPerfMode.DoubleRow
    Exp = mybir.ActivationFunctionType.Exp
    Ident = mybir.ActivationFunctionType.Identity
    mult = mybir.AluOpType.mult
    add = mybir.AluOpType.add

    nc = bacc.Bacc("TRN2", target_bir_lowering=False, debug=False,
                   num_devices=N_CORES)

    # x tensors are host-pre-swizzled to [128, cg, dc, 512] so each
    # column-group load is one dma with 4KB-contiguous per-partition runs
    # (source AND dest), and each cg lands in its OWN tile so the first
    # projection only waits on its own cg's DMA (tile deps are whole-tile)
    xq = nc.dram_tensor("xq_t", [128, 4, 4, 512], f8,
                        kind="ExternalInput").ap()
    xk = nc.dram_tensor("xk_t", [128, 4, 4, 512], f8,
                        kind="ExternalInput").ap()
    # v path stays bf16: near-uniform softmax rows average V down ~30x, so
    # per-entry V quantization noise (~3.6% in fp8) passes straight through
    # to the output; q/k fp8 only costs ~0.4% after the exp
    xv = nc.dram_tensor("xv_t", [128, 4, 4, 512], bf16,
                        kind="ExternalInput").ap()
    wq = nc.dram_tensor("wq", [128, 4 * DG], f8, kind="ExternalInput").ap()
    wk = nc.dram_tensor("wk", [128, 4 * DG], f8, kind="ExternalInput").ap()
    wv = nc.dram_tensor("wv", [128, 4 * DG], bf16,
                        kind="ExternalInput").ap()
    wo = nc.dram_tensor("wo", [128, 2 * D], bf16, kind="ExternalInput").ap()
    # q and k biases in one tensor: [128, (q|k) x pc] — a single tiny DMA
    bqk = nc.dram_tensor("bqk", [128, 4], f32, kind="ExternalInput").ap()
    y = nc.dram_tensor("y", [T, D], bf16, kind="ExternalOutput").ap()

    NT = T // 512        # 4 t/q chunks of 512
    NB = T // 128        # 16 t/k blocks of 128
    ND = D // 128        # 4 contraction chunks over D

    with tile.TileContext(nc) as tc:
        with (
            tc.tile_pool(name="const", bufs=1) as const,
            tc.tile_pool(name="pt", bufs=10) as ppool,
            tc.tile_pool(name="rc", bufs=4) as rcpool,
            tc.tile_pool(name="ysb", bufs=4) as ypool,
            tc.tile_pool(name="psA", bufs=2, space="PSUM") as psA,
            tc.tile_pool(name="psO", bufs=2, space="PSUM") as psO,
        ):
            # ---- input DMAs: per-cg tiles, critical loads first ----------
            def load_w(dram, name, eng, dt):
                t = const.tile([128, ND, DG], dt, tag=name)
                eng.dma_start(
                    out=t[:], in_=dram.rearrange("p (c m) -> p c m", c=ND))
                return t

            def load_x_cg(dram, name, cg, eng, dt):
                # one tile per (tensor, cg): the dependent projections wait
                # only on this one dma (contiguous per-partition runs)
                t = const.tile([128, ND, 512], dt, tag=f"{name}{cg}")
                eng.dma_start(out=t[:], in_=dram[:, cg])
                return t

            def load_x_cg23(dram, name, eng, dt):
                # trailing two column groups in one dma / one tile
                t = const.tile([128, 2, ND, 512], dt, tag=f"{name}23")
                eng.dma_start(out=t[:], in_=dram[:, 2:4])
                return t[:, 0], t[:, 1]

            # the dma issue itself costs the issuing engine ~0.65us, so the
            # startup-critical loads are spread across otherwise-idle
            # engine queues (tensor issues xq0 before its warm-ups, scalar
            # issues wq before its first activation)
            xq_sb = [None] * NT
            xk_sb = [None] * NT
            xv_sb = [None] * NT
            xq_sb[0] = load_x_cg(xq, "xq", 0, nc.scalar, f8)
            xk_sb[0] = load_x_cg(xk, "xk", 0, nc.gpsimd, f8)
            wq_sb = load_w(wq, "wq", nc.scalar, f8)
            wk_sb = load_w(wk, "wk", nc.gpsimd, f8)
            bqk_sb = const.tile([128, 4], f32, tag="bqk")
            nc.sync.dma_start(out=bqk_sb[:], in_=bqk)
            bq_sb = bqk_sb[:, 0:2]
            bk_sb = bqk_sb[:, 2:4]
            wv_sb = load_w(wv, "wv", nc.gpsimd, bf16)
            xv_sb[0] = load_x_cg(xv, "xv", 0, nc.sync, bf16)
            xv_sb[1] = load_x_cg(xv, "xv", 1, nc.sync, bf16)
            xq_sb[1] = load_x_cg(xq, "xq", 1, nc.sync, f8)
            xk_sb[1] = load_x_cg(xk, "xk", 1, nc.gpsimd, f8)
            xq_sb[2], xq_sb[3] = load_x_cg23(xq, "xq", nc.sync, f8)
            xk_sb[2], xk_sb[3] = load_x_cg23(xk, "xk", nc.gpsimd, f8)
            xv_sb[2], xv_sb[3] = load_x_cg23(xv, "xv", nc.sync, bf16)
            wo_sb = const.tile([128, 2, D], bf16, tag="wo")
            nc.sync.dma_start(out=wo_sb[:],
                              in_=wo.rearrange("p (c n) -> p c n", c=2))

            # warm-up during the input-DMA prologue (HAM clock gate)
            warm = const.tile([128, 512], bf16, tag="warm")
            nc.vector.memset(warm[:], 0.0)
            wps = psA.tile([128, 1024], f32, tag="ps", name="wps")
            for _ in range(12):
                nc.tensor.matmul(wps[:, :512], lhsT=warm[:, :128],
                                 rhs=warm[:], start=True, stop=True)
            wexp = const.tile([1, 2], bf16, tag="wexp")
            nc.scalar.activation(wexp[:], warm[0:1, 0:2], Exp)

            # 0/1 lower-triangular for both heads: tri2[k, hh, q] = (q >= k)
            tri2 = const.tile([128, 2, 128], bf16, tag="tri2")
            nc.gpsimd.memset(tri2[:], 1.0)
            for hh in range(2):
                nc.gpsimd.affine_select(
                    out=tri2[:, hh, :], in_=tri2[:, hh, :],
                    compare_op=mybir.AluOpType.is_ge, fill=0.0,
                    base=0, pattern=[[1, 128]], channel_multiplier=-1)

            qT = const.tile([128, 2, T], bf16, tag="qT")
            kT = const.tile([128, 2, T], bf16, tag="kT")
            # PV lhsT = [V_head | 64 ones-cols]: the PE replicates the
            # softmax denominator across PSUM partitions 64..127, so the
            # normalize needs no cross-partition broadcast at all
            vA = const.tile([128, NB, HPG, 2 * HD], bf16, tag="vA")
            nc.vector.memset(vA[:, :, :, HD:], 1.0)
            oTn = const.tile([128, 2, T], bf16, tag="oTn")

            # ---- side-work groups (emitted inside attention j-loops) ----
            def qk2(pc, c):
                # Q^T and K^T chunk c for partition-chunk pc (one 2-bank slot)
                ps = psA.tile([128, 2, 512], f32, tag="ps", name="ps")
                items = ((qT, wq_sb, xq_sb[c], bq_sb),
                         (kT, wk_sb, xk_sb[c], bk_sb))
                for i, (dst, w_sb, x_sb, bsb) in enumerate(items):
                    # fp8 DoubleRow: two 128-row D-chunks contract per matmul
                    for dp in range(ND // 2):
                        nc.tensor.matmul(
                            ps[:, i, :],
                            lhsT=w_sb[:, 2 * dp:2 * dp + 2,
                                      128 * pc:128 * (pc + 1)],
                            rhs=x_sb[:, 2 * dp:2 * dp + 2, :],
                            perf_mode=DR,
                            start=(dp == 0), stop=(dp == ND // 2 - 1),
                            skip_group_check=True)
                # drain on VectorE: one fused (x * 1/WS) + bias per half —
                # keeps the ScalarE queue pure exp
                for i, (dst, w_sb, x_sb, bsb) in enumerate(items):
                    nc.vector.tensor_scalar(
                        out=dst[:, pc, 512 * c:512 * (c + 1)],
                        in0=ps[:, i, :], scalar1=1.0 / WS,
                        scalar2=bsb[:, pc:pc + 1], op0=mult, op1=add)

            def v_quad(tb0):
                # V rows for 4 t-blocks (one 2-bank slot)
                ps = psA.tile([128, 1024], f32, tag="ps", name="ps")
                x_sb = xv_sb[tb0 // 4]
                for i in range(4):
                    for dc in range(ND):
                        nc.tensor.matmul(
                            ps[:, 256 * i:256 * (i + 1)],
                            lhsT=x_sb[:, dc, 128 * i:128 * (i + 1)],
                            rhs=wv_sb[:, dc, :],
                            start=(dc == 0), stop=(dc == ND - 1),
                            skip_group_check=True)
                nc.vector.tensor_copy(
                    vA[:, tb0:tb0 + 4, :, 0:HD],
                    ps.rearrange("p (t h d) -> p t h d", t=4, h=HPG))

            def o_double(tb0, tail=False):
                # output projection for 2 t-blocks (one 2-bank slot)
                yp = psA.tile([128, 1024], f32, tag="ps", name="ps")
                for i in range(2):
                    for pair in range(2):
                        nc.tensor.matmul(
                            yp[:, 512 * i:512 * (i + 1)],
                            lhsT=oTn[:, pair,
                                     128 * (tb0 + i):128 * (tb0 + i + 1)],
                            rhs=wo_sb[:, pair, :],
                            start=(pair == 0), stop=(pair == 1),
                            skip_group_check=True)
                ysb = ypool.tile([128, 1024], bf16, tag="ysb")
                # drain on ScalarE (partition-aligned PSUM read): VectorE
                # carries the rest of the elementwise load
                nc.scalar.activation(ysb[:], yp[:],
                                     mybir.ActivationFunctionType.Copy)
                for i in range(2):
                    nc.sync.dma_start(
                        out=y[128 * (tb0 + i):128 * (tb0 + i + 1), :],
                        in_=ysb[:, 512 * i:512 * (i + 1)])

            # PV runs one exp-step behind, across chunk/pair boundaries:
            # while ScalarE computes exp(j), the PE has S(j+1) AND PV(j-1)
            # queued — the exp stream never waits on a PV drain
            pend_pv = []

            def emit_pv():
                while pend_pv:
                    oT_, a_, j_, pair_, rhs_, first, last = pend_pv.pop(0)
                    for hh in range(2):
                        nc.tensor.matmul(
                            oT_[:, hh, a_:],
                            lhsT=vA[:, j_, 2 * pair_ + hh, :],
                            rhs=rhs_[hh],
                            start=first, stop=last,
                            skip_group_check=True)
                        # partitions 0..63: O^T rows; 64..127: the softmax
                        # denominator replicated 64x (ones-cols of vA)

            # ---- attention ------------------------------------------------
            def attention_qc(pair, qc, side):
                """Emit one q-chunk of attention. Returns deferred normalize
                closures (one per head) — emitted later, inside the NEXT
                chunk's j-loop, so their wait on the rc broadcast DMA never
                head-of-line-blocks the Vector queue (which would stall psA
                slot releases and starve ScalarE)."""
                q0 = 512 * qc
                n_j = 4 * qc + 4
                oT = psO.tile([128, 2, 512], f32, tag="oT", name="oT")
                # diagonal windows first: their exp->mask->PV chain crosses
                # an extra engine, so keep it off the chunk tail where it
                # would delay the next chunk's S matmuls in the PE FIFO
                order = list(range(4 * qc, n_j)) + list(range(4 * qc))

                for idx, j in enumerate(order):
                    a = max(0, 128 * j - q0)
                    sT = psA.tile([128, 2, 512], f32, tag="ps", name="sT")
                    for hh in range(2):
                        o = 64 * hh
                        nc.tensor.matmul(
                            sT[:, hh, a:],
                            lhsT=kT[o:o + 64, pair, 128 * j:128 * (j + 1)],
                            rhs=qT[o:o + 64, pair, q0 + a:q0 + 512],
                            start=True, stop=True, skip_group_check=True)
                    sch = 128 * j < q0 and j % 3 == 1
                    if sch:
                        # Schraudolph exp2: bf16 bits of 2^t are linear in t
                        # (+-3% after the balancing shift) — one VectorE op
                        # offloads this block's exp from busy ScalarE
                        pTi = ppool.tile([128, 2, 512], mybir.dt.int16,
                                         tag="pt")
                        nc.vector.tensor_scalar(
                            out=pTi[:], in0=sT[:], scalar1=SCH_A,
                            scalar2=SCH_B, op0=mult, op1=add)
                        pv_rhs = [pTi[:, hh, :].bitcast(bf16)
                                  for hh in range(2)]
                    else:
                        pT = ppool.tile([128, 2, 512], bf16, tag="pt")
                        nc.scalar.activation(pT[:, :, a:], sT[:, :, a:], Exp,
                                             scale=SM_SCALE)
                        if 128 * j >= q0:  # diagonal window, both heads
                            nc.gpsimd.tensor_tensor(
                                pT[:, :, a:a + 128], pT[:, :, a:a + 128],
                                tri2[:], mult)
                        pv_rhs = [pT[:, hh, a:] for hh in range(2)]
                    emit_pv()
                    pend_pv.append((oT, a, j, pair, pv_rhs,
                                    idx == 0, idx == n_j - 1))
                    if idx < len(side) and side[idx] is not None:
                        side[idx]()
                emit_pv()  # drain this chunk's last PV before its boundary
                # softmax denominator work is ALL deferred into the next
                # chunk's j-loop: the reciprocal waits on this chunk's last
                # PV matmul — emitted at a boundary it would head-of-line-
                # block the Vector queue, stalling psA slot releases and
                # starving ScalarE.
                rc = rcpool.tile([64, 2, 512], f32, tag="rc")

                def fin0():
                    # denominators sit pre-replicated on PSUM partitions
                    # 64..127 (ones-block of vA). Partition-shifted copy on
                    # the plain ALU path (custom-DVE ops and mismatched
                    # in0/in1 offsets don't shift on hw), then reciprocal.
                    # Keeping both on VectorE measured faster than a ScalarE
                    # copy (which delays the exp stream at chunk starts).
                    nc.vector.tensor_copy(rc[:], oT[HD:, :, :])
                    nc.vector.reciprocal_approx_fast(rc[:], rc[:])

                def norm(hh):
                    nc.vector.tensor_tensor(
                        oTn[64 * hh:64 * hh + 64, pair, q0:q0 + 512],
                        oT[0:HD, hh, :], rc[:, hh, :], mult)
                return [fin0, lambda: norm(0), lambda: norm(1)]

            def place(n_j, assign):
                side = [None] * n_j
                for slot, fn in assign:
                    assert side[slot] is None and slot < n_j
                    side[slot] = fn
                return side

            pend = []  # deferred boundary closures from the previous qc

            def run_qc(pair, qc, extras):
                # prev chunk's srow/recip/DMA at slot 0, normalizes at 4,6
                # (2,3 when the chunk is short); extras carry explicit slots
                nonlocal pend
                n_j = 4 * qc + 4
                assign = []
                if pend:
                    f0, n0, n1 = pend
                    if n_j >= 8:
                        assign += [(0, f0), (4, n0), (6, n1)]
                    else:
                        assign += [(0, f0), (2, n0), (3, n1)]
                assign += extras
                pend = attention_qc(pair, qc, place(n_j, assign))

            qk2(0, 0)
            v_quad(0)
            run_qc(0, 0, [(0, lambda: v_quad(4)), (1, lambda: qk2(0, 1))])
            run_qc(0, 1, [(1, lambda: qk2(0, 2)), (3, lambda: v_quad(8)),
                          (5, lambda: qk2(1, 0))])
            run_qc(0, 2, [(1, lambda: qk2(0, 3)), (3, lambda: v_quad(12)),
                          (5, lambda: qk2(1, 1)), (7, lambda: qk2(1, 2))])
            run_qc(0, 3, [(1, lambda: qk2(1, 3))])
            run_qc(1, 0, [])
            run_qc(1, 1, [])
            run_qc(1, 2, [(1, lambda: o_double(0)), (3, lambda: o_double(2)),
                          (8, lambda: o_double(4)),
                          (10, lambda: o_double(6))])
            run_qc(1, 3, [(8, lambda: o_double(8)),
                          (10, lambda: o_double(10))])
            emit_pv()
            for fn in pend:
                fn()
            o_double(12, tail=True)
            o_double(14)

    nc.compile()
    return nc


def _get_compiled():
    global _compiled
    if _compiled is None:
        _compiled = _build()
    return _compiled


def _reference_fallback(queries, keys, values, mask, Wq, bq, Wk, bk, Wv, bv,
                        Wo, bo):
    def split_heads(x):
        b, t, c = x.shape
        return x.reshape(b, t, H, c // H).transpose(0, 2, 1, 3)

    q = split_heads(queries @ Wq + bq)
    k = split_heads(keys @ Wk + bk)
    v = split_heads(values @ Wv + bv)
    wei = np.einsum("bhqd,bhkd->bhqk", q, k) * SM_SCALE
    wei = wei + (-1e9) * mask
    wei = wei - wei.max(axis=-1, keepdims=True)
    wei = np.exp(wei)
    wei = wei / wei.sum(axis=-1, keepdims=True)
    out = np.einsum("bhqk,bhkd->bhqd", wei, v)
    out = out.transpose(0, 2, 1, 3).reshape(queries.shape[0],
                                            queries.shape[1], D)
    return (out @ Wo + bo).astype(np.float32)


def _xprep(x, dtype):
    """[T, D] -> transposed [128, cg, dc, 512]: per (partition, cg) the
    4 dc-chunks are contiguous DMA runs."""
    xt = np.asarray(x).T  # [D, T]
    return np.ascontiguousarray(
        xt.reshape(4, 128, 4, 512).transpose(1, 2, 0, 3)
    ).astype(dtype)


def _wprep(w, scale, dtype):
    """[D, DG] -> partition-major [128, ND*DG] in `dtype`, pre-scaled."""
    w = np.asarray(w) * scale
    n = w.shape[1]
    return np.ascontiguousarray(
        w.reshape(-1, 128, n).swapaxes(0, 1).reshape(128, -1)).astype(dtype)


def make_in_maps(queries, keys, values, Wq, bq, Wk, bk, Wv, Wo):
    in_maps = []
    for core in range(N_CORES):
        b, g = core // 2, core % 2
        sl = slice(g * DG, (g + 1) * DG)
        in_maps.append({
            "xq_t": _xprep(queries[b], _F8),
            "xk_t": _xprep(keys[b], _F8),
            "xv_t": _xprep(values[b], _BF16),
            "wq": _wprep(Wq[:, sl], WS, _F8),
            "wk": _wprep(Wk[:, sl], WS, _F8),
            "wv": _wprep(Wv[:, sl], 1.0, _BF16),
            "wo": _wprep(Wo[sl, :], 1.0, _BF16),
            "bqk": np.ascontiguousarray(np.concatenate([
                np.asarray(bq, np.float32)[sl].reshape(2, 128).T,
                np.asarray(bk, np.float32)[sl].reshape(2, 128).T,
            ], axis=1)),
        })
    return in_maps


def kernel(queries, keys, values, mask, Wq, bq, Wk, bk, Wv, bv, Wo, bo):
    queries = np.asarray(queries, np.float32)
    keys = np.asarray(keys, np.float32)
    values = np.asarray(values, np.float32)
    Wq, Wk, Wv, Wo = (np.asarray(w, np.float32) for w in (Wq, Wk, Wv, Wo))
    bq, bk, bv, bo = (np.asarray(v_, np.float32) for v_ in (bq, bk, bv, bo))
    mask2d = np.asarray(mask, np.float32).reshape(T, T)
    causal = np.triu(np.ones((T, T), np.float32), k=1)
    if not np.array_equal(mask2d, causal):
        return _reference_fallback(queries, keys, values,
                                   np.asarray(mask, np.float32),
                                   Wq, bq, Wk, bk, Wv, bv, Wo, bo)

    from concourse.bass_utils import run_bass_kernel_spmd

    nc = _get_compiled()
    in_maps = make_in_maps(queries, keys, values, Wq, bq, Wk, bk, Wv, Wo)
    res = run_bass_kernel_spmd(nc, in_maps, list(range(N_CORES)))
    out = np.zeros((B, T, D), np.float32)
    for core in range(N_CORES):
        out[core // 2] += res.results[core]["y"].astype(np.float32)
    out += bo + bv @ Wo   # value bias is separable (softmax rows sum to 1)
    return out

